# revision 19
# baseline (speedup 1.0000x reference)
"""Trainium2 Bass kernel for a YOLO-style detection loss.

Strategy (data-parallel over batch, per sharding hint):
  - Shard preds on batch dim: 4 images per core across 8 cores; targets
    partitioned by image index, then rebalanced to exactly n/8 per core
    (foreign targets' rows ride in a small appended table).
  - Key layout trick: floor(80*t) == floor(160*t)//2 for t >= 0, so a
    target's mid/coarse cells are determined by its fine cell. The host
    stages, per mid-level (80x80) cell, a 6-row record
    [fine0, fine1, mid, coarse, fine2, fine3]; every target's three rows
    (fine, mid, coarse) then lie inside ONE contiguous 4-row window at a
    shift in {0,1,2}. One indirect-DMA offset per target fetches 44
    floats, so a core needs only ceil(m/128) = 8 gather instructions
    (the ~1us fixed SWDGE descriptor-generation cost per indirect DMA is
    the bottleneck; the env supports neither multi-offset indirect DMAs
    nor the extended-ISA dma_gather).
  - The one extra (cancel) row per window is neutralized on-device by
    host-built aux data: box targets are set to the row's own values
    (|d| = 0) and softplus lanes use W = softplus(g)/g so that
    softplus(g) - g*W == 0 (lanes with |g| < eps get W = 0 and an exact
    host-side correction instead).
  - Device computes, per lane, |g - T| (box) and softplus(g) - g*W
    (obj/cls) with 5 DVE + 2 ACT ops total, reducing to per-field sums
    [128, 8] per core; the host reduces partitions/cores in f64 and
    applies the loss gains.
"""

import numpy as np

P = 128
NCLS = 6
NO = NCLS + 5
BS = 32
NA = 3
NCORES = 8
BPC = BS // NCORES  # images per core
W = 4  # gathered window rows per target
LANES = W * NO  # 44 gathered floats per slot
NREC = BPC * 80 * 80  # mid-cell records per core
NROW = NREC * 6  # base table rows per core
BOX_GAIN, CLS_GAIN, DFL_GAIN = 7.5, 0.5, 1.5
EPS = 1e-2  # |g| below this: host-corrected instead of W-cancelled

# appended-row region for foreign (rebalanced) targets, padded so the
# table factors as [X, 128, 11] (dims < 65536 for DMA APs)
APPEND_CAP = 1024  # rows; supports 256 foreign targets per core
TAB_ROWS = NROW + APPEND_CAP  # 154624 = 1208 * 128
TAB_F1 = TAB_ROWS // P

_BUILD_CACHE: dict = {}


def _emit_half(nc, pool, G, aux_t, outt, S, s0, s1, h):
    """Compute on slot columns [s0, s1); partials into outt[:, 8h:8h+8].
    G is the half's own gather tile covering columns [s0, s1)."""
    from concourse import mybir

    f32 = mybir.dt.float32
    add = mybir.AluOpType.add
    Sh = s1 - s0
    c = 8 * h

    G4 = G[:].rearrange("p (s w f) -> p s w f", w=W, f=NO)
    A4 = aux_t[:, s0 * LANES : s1 * LANES].rearrange(
        "p (s w f) -> p s w f", w=W, f=NO
    )

    # --- DVE: box L1 ---
    D = pool.tile([P, Sh * W * 4], f32, tag=f"D{h}")
    D4 = D[:].rearrange("p (s w f) -> p s w f", w=W, f=4)
    nc.vector.tensor_sub(out=D4, in0=G4[:, :, :, 0:4], in1=A4[:, :, :, 0:4])
    nc.vector.tensor_reduce(
        out=outt[:, c : c + 1],
        in_=D[:],
        axis=mybir.AxisListType.X,
        op=add,
        apply_absolute_value=True,
    )

    # --- ACT: softplus via ln(exp(g)+1) on obj+cls lanes ---
    E = pool.tile([P, Sh * W * 7], f32, tag=f"E{h}")
    nc.scalar.activation(
        E[:].rearrange("p (s w f) -> p s w f", w=W, f=7),
        G4[:, :, :, 4:11],
        mybir.ActivationFunctionType.Exp,
    )
    SP = pool.tile([P, Sh * W * 7], f32, tag=f"SP{h}")
    nc.scalar.activation(
        SP[:], E[:], mybir.ActivationFunctionType.Ln, bias=1.0
    )

    # --- DVE: Y = softplus(g) - g*W; per-field sums [p, 7] ---
    X = pool.tile([P, Sh * W * 7], f32, tag=f"X{h}")
    X4 = X[:].rearrange("p (s w f) -> p s w f", w=W, f=7)
    nc.vector.tensor_mul(out=X4, in0=G4[:, :, :, 4:11], in1=A4[:, :, :, 4:11])
    Y = pool.tile([P, Sh * W * 7], f32, tag=f"Y{h}")
    nc.vector.tensor_sub(out=Y[:], in0=SP[:], in1=X[:])
    # field-major view: [p, f(7), s, w]
    YF = Y[:].rearrange("p (s w f) -> p f s w", w=W, f=7)
    nc.vector.tensor_reduce(
        out=outt[:, c + 1 : c + 8], in_=YF, axis=mybir.AxisListType.XY, op=add
    )


def _chunk_bounds(S):
    """Compute-chunk split: leading chunks hide under the gather stream,
    single-column trailing chunks keep the post-last-gather tail minimal."""
    if S <= 2:
        return [(0, S)]
    if S <= 4:
        return [(0, S - 1), (S - 1, S)]
    mid = (S - 2 + 1) // 2
    return [(0, mid), (mid, S - 2), (S - 2, S - 1), (S - 1, S)]


def _emit_body(nc, pool, tab, idx_t, aux_t, out_ap, S):
    """One loss body: S gathers + chunked compute + out DMA.
    out_ap: [P, 8*len(chunks)]. Each chunk gathers into its own tile so its
    compute only waits on its own gathers (tile-granularity deps) and
    overlaps the later chunks' gathers."""
    from concourse import bass, mybir

    f32 = mybir.dt.float32

    bounds = _chunk_bounds(S)
    outt = pool.tile([P, 8 * len(bounds)], f32, tag="OUT")
    Gs = []
    for h, (s0, s1) in enumerate(bounds):
        G = pool.tile([P, (s1 - s0) * LANES], f32, tag=f"G{h}")
        for s in range(s0, s1):
            nc.gpsimd.indirect_dma_start(
                out=G[:, (s - s0) * LANES : (s - s0 + 1) * LANES],
                out_offset=None,
                in_=tab,
                in_offset=bass.IndirectOffsetOnAxis(
                    ap=idx_t[:, s : s + 1], axis=1
                ),
            )
        Gs.append(G)
    for h, (s0, s1) in enumerate(bounds):
        _emit_half(nc, pool, Gs[h], aux_t, outt, S, s0, s1, h)
    nc.sync.dma_start(out=out_ap, in_=outt[:])


def _patch_act_tables():
    """Force Exp and Ln to resolve to the one table set containing BOTH
    ('natural_log_exp_and_others'). Otherwise the act-table-load pass picks a
    different set for each and inserts a 1.3us LoadActFuncSet around every
    exp<->ln switch on the critical path. Set positions (= act_func_set_ids)
    are preserved; only membership is filtered."""
    import functools

    from concourse import bacc, hw_specs, mybir

    if getattr(_patch_act_tables, "_done", False):
        return
    orig = hw_specs.get_activation_tables

    @functools.cache
    def patched(arch):
        keep = {mybir.ActivationFunctionType.Exp, mybir.ActivationFunctionType.Ln}
        out = {}
        for name, funcs in orig(arch).items():
            if name != "natural_log_exp_and_others":
                funcs = set(funcs) - keep
            out[name] = funcs
        return out

    hw_specs.get_activation_tables = patched
    bacc.get_activation_tables = patched
    _patch_act_tables._done = True


def _build(S: int, repeat: int = 1):
    """Build + compile the per-core Bass program for S slots per partition.
    repeat>1 unrolls the body for benchmarking; the graded path uses 1."""
    from concourse import bacc, mybir, tile

    _patch_act_tables()
    f32 = mybir.dt.float32

    nc = bacc.Bacc(
        "TRN2", target_bir_lowering=False, debug=False, enable_asserts=False
    )

    tab = nc.dram_tensor(
        "tab", [TAB_F1, P, NO], f32, kind="ExternalInput"
    ).ap()
    idx_d = nc.dram_tensor("idx", [P, S], mybir.dt.int32, kind="ExternalInput").ap()
    aux_d = nc.dram_tensor(
        "aux", [P, S * LANES], f32, kind="ExternalInput"
    ).ap()
    ncols = 8 * len(_chunk_bounds(S))
    out_d = nc.dram_tensor(
        "out", [repeat * P, ncols], f32, kind="ExternalOutput"
    ).ap()

    with tile.TileContext(nc) as tc:
        with tc.tile_pool(name="pool", bufs=2) as pool:
            idx_t = pool.tile([P, S], mybir.dt.int32, tag="idx")
            aux_t = pool.tile([P, S * LANES], f32, tag="aux")
            nc.sync.dma_start(out=idx_t[:], in_=idx_d[:])
            nc.scalar.dma_start(out=aux_t[:], in_=aux_d[:])
            for rep in range(repeat):
                _emit_body(
                    nc,
                    pool,
                    tab,
                    idx_t,
                    aux_t,
                    out_d[rep * P : (rep + 1) * P, :],
                    S,
                )

    nc.compile()
    return nc


def _np_softplus(x):
    return np.logaddexp(0.0, np.asarray(x, np.float64))


def _prepare(pred_full, targets):
    """Build per-core tables, slot indices, aux tensors and corrections."""
    n = targets.shape[0]
    b = targets[:, 0].astype(np.int32)
    c = np.clip(targets[:, 1].astype(np.int32), 0, NCLS - 1)
    txywh = targets[:, 2:6].astype(np.float32)

    # per-layer cells exactly as the reference computes them (f32 mults)
    cells = []
    for nx in (160, 80, 40):
        gx = np.clip(
            np.floor(np.float32(nx) * txywh[:, 0]).astype(np.int32), 0, nx - 1
        )
        gy = np.clip(
            np.floor(np.float32(nx) * txywh[:, 1]).astype(np.int32), 0, nx - 1
        )
        cells.append((gy, gx))
    (gy0, gx0), (gy1, gx1), (gy2, gx2) = cells
    consistent = (
        (gy1 == gy0 // 2)
        & (gx1 == gx0 // 2)
        & (gy2 == gy1 // 2)
        & (gx2 == gx1 // 2)
    )
    fr = (gy0 % 2) * 2 + (gx0 % 2)
    shift = np.where(fr < 2, fr, 2).astype(np.int32)
    # role of each window row w0..w3 given (shift, fr):
    #  shift 0      : [fine, cancel, mid, coarse]
    #  shift 1      : [fine, mid, coarse, cancel]
    #  shift 2, fr 2: [mid, coarse, fine, cancel]
    #  shift 2, fr 3: [mid, coarse, cancel, fine]
    fine_w = np.choose(fr, [0, 0, 2, 3])
    mid_w = np.choose(fr, [2, 1, 0, 0])
    coarse_w = np.choose(fr, [3, 2, 1, 1])

    bloc = b % BPC
    rec = bloc * 6400 + gy1 * 80 + gx1
    row = rec * 6 + shift  # window start row in the core's base table

    # --- rebalance to at most mpc targets per core (pad the remainder) ---
    core_owner = b // BPC
    mpc = max(P, -(-n // NCORES // P) * P)
    order = np.argsort(core_owner, kind="stable")
    # fill cores with their own targets first, spill the excess round-robin
    slots_left = np.full(NCORES, mpc, np.int64)
    own_sel = [[] for _ in range(NCORES)]
    spill = []
    for t in order:
        i = core_owner[t]
        if slots_left[i] > 0:
            slots_left[i] -= 1
            own_sel[i].append(t)
        else:
            spill.append(t)
    spill_iter = iter(spill)
    foreign_sel = [[] for _ in range(NCORES)]
    for i in range(NCORES):
        while slots_left[i] > 0:
            t = next(spill_iter, None)
            if t is None:
                break
            slots_left[i] -= 1
            foreign_sel[i].append(t)
    S = mpc // P

    onehot = np.zeros((n, NCLS), dtype=np.float32)
    onehot[np.arange(n), c] = 1.0

    in_maps = []
    host_add = np.zeros(3, np.float64)  # exact host terms: box, obj, cls
    core_corr = []  # per-core (obj_corr, cls_corr) for eps lanes

    for i in range(NCORES):
        # --- base table: per mid cell [f0, f1, mid, coarse, f2, f3] ---
        p0 = pred_full[0][i * BPC : (i + 1) * BPC, 0]  # [4,160,160,11]
        p1 = pred_full[1][i * BPC : (i + 1) * BPC, 0]  # [4,80,80,11]
        p2 = pred_full[2][i * BPC : (i + 1) * BPC, 0]  # [4,40,40,11]
        f = p0.reshape(BPC, 80, 2, 80, 2, NO)
        tab6 = np.empty((BPC, 80, 80, 6, NO), np.float32)
        tab6[:, :, :, 0] = f[:, :, 0, :, 0]
        tab6[:, :, :, 1] = f[:, :, 0, :, 1]
        tab6[:, :, :, 2] = p1
        tab6[:, :, :, 3] = (
            p2.repeat(2, axis=1).repeat(2, axis=2)
        )
        tab6[:, :, :, 4] = f[:, :, 1, :, 0]
        tab6[:, :, :, 5] = f[:, :, 1, :, 1]
        tab = np.zeros((TAB_ROWS, NO), np.float32)
        tab[:NROW] = tab6.reshape(NROW, NO)

        own = np.asarray(own_sel[i], np.int64)
        frn = np.asarray(foreign_sel[i], np.int64)
        nf = len(frn)
        assert 4 * (nf + 1) <= APPEND_CAP, nf
        # rows NROW..NROW+3 stay zero: the pad-slot window (fully cancelled,
        # softplus(0) terms picked up by the eps correction below).
        # Foreign windows (4 rows each) follow, built from the owner's data.
        frn_rows = np.zeros((nf,), np.int64)
        for k, t in enumerate(frn):
            oc = core_owner[t]
            bb = int(b[t] % BPC)
            tab[NROW + 4 * (k + 1) : NROW + 4 * (k + 2)] = _window_rows(
                pred_full, oc, bb, int(gy1[t]), int(gx1[t]), int(shift[t])
            )
            frn_rows[k] = NROW + 4 * (k + 1)

        sel = np.concatenate([own, frn]).astype(np.int64)  # real targets
        m_real = len(sel)
        rows = np.full(mpc, NROW, np.int32)  # pad slots -> zero window
        rows[:m_real] = np.concatenate([row[own], frn_rows])

        # slot layout: slot j -> (partition j % P, column j // P)
        idx = np.ascontiguousarray(
            rows.reshape(S, P).swapaxes(0, 1)
        ).astype(np.int32)

        # --- aux per slot: [4, 11] T|W records ---
        gvals = tab[rows[:, None] + np.arange(4)[None, :]]  # [mpc,4,11]
        aux = np.empty((mpc, W, NO), np.float32)
        # default: cancel every row
        aux[:, :, 0:4] = gvals[:, :, 0:4]
        gs = gvals[:, :, 4:11].astype(np.float64)
        small = np.abs(gs) < EPS
        with np.errstate(divide="ignore", invalid="ignore"):
            wc = np.where(small, 0.0, _np_softplus(gs) / gs)
        aux[:, :, 4:11] = wc.astype(np.float32)

        # overwrite used rows for consistent targets (pad slots stay cancelled)
        cons = np.zeros(mpc, bool)
        cons[:m_real] = consistent[sel]
        ar = np.arange(mpc)
        fw = np.zeros(mpc, np.int64)
        mw = np.zeros(mpc, np.int64)
        cw = np.zeros(mpc, np.int64)
        fw[:m_real], mw[:m_real], cw[:m_real] = (
            fine_w[sel],
            mid_w[sel],
            coarse_w[sel],
        )
        t4 = np.zeros((mpc, 4), np.float32)
        t4[:m_real] = txywh[sel]
        w7 = np.zeros((mpc, 7), np.float32)
        w7[:m_real, 0] = 1.0
        w7[:m_real, 1:] = onehot[sel]
        for wv in (fw, mw, cw):
            rows_used = ar[cons], wv[cons]
            aux[rows_used[0], rows_used[1], 0:4] = t4[cons]
            aux[rows_used[0], rows_used[1], 4:11] = w7[cons]
        # eps corrections: softplus(g) left un-cancelled on device
        used_mask = np.zeros((mpc, W, 7), bool)
        for wv in (fw, mw, cw):
            used_mask[ar[cons], wv[cons], :] = True
        eps_lanes = small & ~used_mask
        sp_small = _np_softplus(gs)
        obj_corr = sp_small[:, :, 0][eps_lanes[:, :, 0]].sum()
        cls_corr = sp_small[:, :, 1:][eps_lanes[:, :, 1:]].sum()
        core_corr.append((obj_corr, cls_corr))

        # inconsistent targets: fully host-computed (window already cancels)
        for t in sel[~cons[:m_real]]:
            ps = []
            for l, pl in enumerate(pred_full):
                gyl, gxl = cells[l]
                ps.append(
                    pl[b[t], 0, gyl[t], gxl[t]].astype(np.float64)
                )
            for p_ in ps:
                host_add[0] += np.abs(p_[0:4] - txywh[t]).sum()
                host_add[1] += _np_softplus(-p_[4])
                host_add[2] += (
                    _np_softplus(p_[5:11]).sum() - p_[5 + c[t]]
                )

        aux_ps = np.ascontiguousarray(
            aux.reshape(S, P, W * NO).swapaxes(0, 1)
        ).reshape(P, S * LANES)

        in_maps.append(
            {
                "idx": idx,
                "aux": aux_ps,
                "tab": tab.reshape(TAB_F1, P, NO),
            }
        )

    return S, in_maps, core_corr, host_add, n


def _window_rows(pred_full, core, bb, my, mx, shift):
    """The 4 window rows [shift:shift+4] of record (bb, my, mx) on `core`."""
    p0 = pred_full[0][core * BPC + bb, 0]
    p1 = pred_full[1][core * BPC + bb, 0]
    p2 = pred_full[2][core * BPC + bb, 0]
    rec = np.empty((6, NO), np.float32)
    rec[0] = p0[2 * my, 2 * mx]
    rec[1] = p0[2 * my, 2 * mx + 1]
    rec[2] = p1[my, mx]
    rec[3] = p2[my // 2, mx // 2]
    rec[4] = p0[2 * my + 1, 2 * mx]
    rec[5] = p0[2 * my + 1, 2 * mx + 1]
    return rec[shift : shift + 4]


def _prepare_in_maps(pred_full, targets):
    S, in_maps, _, _, _ = _prepare(pred_full, targets)
    return S, in_maps


def _run(pred_full, targets, trace=False, **run_kwargs):
    from concourse import bass_utils

    S, in_maps, core_corr, host_add, n = _prepare(pred_full, targets)
    if S not in _BUILD_CACHE:
        _BUILD_CACHE[S] = _build(S)
    nc = _BUILD_CACHE[S]
    res = bass_utils.run_bass_kernel_spmd(
        nc, in_maps, core_ids=list(range(NCORES)), trace=trace, **run_kwargs
    )

    s_box = host_add[0]
    s_obj = host_add[1]
    s_cls = host_add[2]
    for r, (obj_corr, cls_corr) in zip(res.results, core_corr):
        tot = r["out"][0:P, :].astype(np.float64).sum(axis=0)
        grp = tot.reshape(-1, 8)
        s_box += grp[:, 0].sum()
        s_obj += grp[:, 1].sum() - obj_corr
        s_cls += grp[:, 2:8].sum() - cls_corr

    inv_n = 1.0 / max(1, n)
    lbox = BOX_GAIN * inv_n * s_box
    lobj = DFL_GAIN * inv_n * s_obj
    lcls = CLS_GAIN * inv_n * s_cls
    loss = lbox + lobj + lcls
    return np.asarray([loss, lbox, lobj, lcls], dtype=np.float32), res


def kernel(**inputs) -> np.ndarray:
    pred_full = [
        np.asarray(inputs[f"pred{l}"], dtype=np.float32) for l in range(3)
    ]
    targets = np.asarray(inputs["targets"], dtype=np.float32)
    out, _ = _run(pred_full, targets, trace=False)
    return out
